# revision 6
# baseline (speedup 1.0000x reference)
"""CBAM attention module (channel gate + spatial softmax attention) on 8 TRN2
NeuronCores, data-parallel over the batch dimension.

Reference computation (per sample b):
    m  = mean_n x[c, n];  mx = max_n x[c, n]
    gate = sigmoid(w2 @ (relu(w1 @ m) + relu(w1 @ mx)))          # (C,)
    x1 = gate[:, None] * x
    s  = sw0 * max_c x1 + sw1 * mean_c x1                        # (N,)
    s  = relu(A * s + Bconst)        # BatchNorm1d(1) eval, folded on host
    att = softmax_n(s)
    out = att[None, :] * x1

Kernel structure per core (2 samples each):
    pass 1: stream x, per-(c) sum over n (ScalarE activation accum) and
            max over n (VectorE reduce) -> tiny MLP on TensorE -> gate
    pass 2: stream x again; ScalarE scales by gate -> x1; VectorE max-combines
            the 8 c-chunks; TensorE transposes 128x128 blocks so VectorE can
            reduce over c; TensorE matvec (gate weights) accumulates sum over
            c in PSUM.  Softmax over n in the transposed layout.
    pass 3: stream x again; out = (x * att) * gate, write back.

x is read 3x and written 1x (the algorithm's lower bound: gate needs a full
pass over n, att needs a full pass over c of x1) -> memory-bound at
~512 MiB per core.
"""

import os
import numpy as np

B, C, N, RATIO = 16, 1024, 16384, 8
H = C // RATIO  # 128
BN_EPS = 1e-5
N_CORES = 8
BC = B // N_CORES  # samples per core

_cached = {}


def _build_nc(NT=512, BC=BC, C=C, N=N, H=H):
    import concourse.bass as bass
    import concourse.bacc as bacc
    import concourse.mybir as mybir
    import concourse.tile as tile
    import concourse.bass_isa as bass_isa
    from concourse import masks
    from contextlib import ExitStack

    f32 = mybir.dt.float32
    AF = mybir.ActivationFunctionType
    X = mybir.AxisListType.X

    K = C // 128          # 8 c-chunks
    NJ = N // NT          # n-tiles per sample
    NB = N // 128         # 128-blocks per sample (transpose-layout columns)
    BPT = NT // 128       # 128-blocks per n-tile
    MV = max(1, NT // 512)  # matvec pieces per n-tile (moving free dim <= 512)
    MVW = min(NT, 512)

    nc = bacc.Bacc("TRN2", target_bir_lowering=False, debug=False,
                   num_devices=N_CORES)

    x = nc.dram_tensor("x", (BC, C, N), f32, kind="ExternalInput").ap()
    w1t = nc.dram_tensor("w1t", (C, H), f32, kind="ExternalInput").ap()
    w2t = nc.dram_tensor("w2t", (H, C), f32, kind="ExternalInput").ap()
    # params = [sw0, sw1/C, A, Bconst]
    params = nc.dram_tensor("params", (1, 4), f32, kind="ExternalInput").ap()
    out = nc.dram_tensor("out", (BC, C, N), f32, kind="ExternalOutput").ap()

    att_dram = nc.dram_tensor("att_scratch", (BC, N), f32, kind="Internal").ap()
    cm_dram = nc.dram_tensor("cm_scratch", (BC, N), f32, kind="Internal").ap()

    with tile.TileContext(nc) as tc, ExitStack() as ctx:
        consts = ctx.enter_context(tc.tile_pool(name="consts", bufs=1))
        big = ctx.enter_context(tc.tile_pool(name="big", bufs=3))
        small = ctx.enter_context(tc.tile_pool(name="small", bufs=3))
        psum = ctx.enter_context(tc.tile_pool(name="psum", bufs=2, space="PSUM"))

        # ---- constants ----
        identity = consts.tile([128, 128], f32)
        masks.make_identity(nc, identity)
        params_sb = consts.tile([128, 4], f32)
        nc.sync.dma_start(out=params_sb, in_=params.to_broadcast((128, 4)))
        w1t_sb = consts.tile([128, K, H], f32)
        nc.sync.dma_start(out=w1t_sb, in_=w1t.rearrange("(k p) h -> p k h", p=128))
        w2t_sb = consts.tile([H, C], f32)
        nc.sync.dma_start(out=w2t_sb, in_=w2t)

        # ---- persistent stats ----
        mx_cols = consts.tile([128, BC, K, NJ], f32)
        sum_cols = consts.tile([128, BC, K, NJ], f32)
        stats = consts.tile([128, K, 2 * BC], f32)   # cols: [sum_b]*BC + [max_b]*BC
        gate_sb = consts.tile([128, K, BC], f32)
        cx_t = consts.tile([128, BC, NB], f32)

        # ================= pass 1: per-channel sum & max over n ============
        for b in range(BC):
            xr = x[b].rearrange("(k p) n -> p k n", p=128)
            for j in range(NJ):
                xin = big.tile([128, K, NT], f32, tag="xin")
                nc.sync.dma_start(out=xin, in_=xr[:, :, j * NT:(j + 1) * NT])
                nc.vector.reduce_max(out=mx_cols[:, b, :, j:j + 1], in_=xin,
                                     axis=X)
                dummy = big.tile([128, K, NT], f32, tag="big2")
                for k in range(K):
                    nc.scalar.activation(
                        out=dummy[:, k, :], in_=xin[:, k, :], func=AF.Copy,
                        accum_out=sum_cols[:, b, k, j:j + 1])

        # ================= MLP -> gate ====================================
        for b in range(BC):
            nc.vector.reduce_sum(out=stats[:, :, b:b + 1],
                                 in_=sum_cols[:, b, :, :], axis=X)
            nc.vector.reduce_max(out=stats[:, :, BC + b:BC + b + 1],
                                 in_=mx_cols[:, b, :, :], axis=X)

        h_psum = psum.tile([H, 2 * BC], f32, tag="ps")
        for k in range(K):
            nc.tensor.matmul(h_psum, lhsT=w1t_sb[:, k, :], rhs=stats[:, k, :],
                             start=(k == 0), stop=(k == K - 1))
        hr = small.tile([H, 2 * BC], f32, tag="hr")
        nc.scalar.activation(out=hr[:, 0:BC], in_=h_psum[:, 0:BC],
                             func=AF.Relu, scale=1.0 / N)
        nc.scalar.activation(out=hr[:, BC:2 * BC], in_=h_psum[:, BC:2 * BC],
                             func=AF.Relu, scale=1.0)
        hsum = small.tile([H, BC], f32, tag="hsum")
        nc.vector.tensor_add(out=hsum, in0=hr[:, 0:BC], in1=hr[:, BC:2 * BC])
        for k in range(K):
            g_psum = psum.tile([128, BC], f32, tag="ps")
            nc.tensor.matmul(g_psum, lhsT=w2t_sb[:, k * 128:(k + 1) * 128],
                             rhs=hsum, start=True, stop=True)
            nc.scalar.activation(out=gate_sb[:, k, :], in_=g_psum,
                                 func=AF.Sigmoid)

        # ====== pass 2: x1 = gate*x; max/sum over c; softmax over n =======
        for b in range(BC):
            xr = x[b].rearrange("(k p) n -> p k n", p=128)
            for j in range(NJ):
                xin = big.tile([128, K, NT], f32, tag="xin")
                nc.sync.dma_start(out=xin, in_=xr[:, :, j * NT:(j + 1) * NT])
                x1 = big.tile([128, K, NT], f32, tag="big2")
                for k in range(K):
                    nc.scalar.activation(out=x1[:, k, :], in_=xin[:, k, :],
                                         func=AF.Copy,
                                         scale=gate_sb[:, k, b:b + 1])
                # sum over c: matvec with gate weights, accumulated in PSUM
                for h2 in range(MV):
                    cm_psum = psum.tile([1, MVW], f32, tag="ps")
                    for k in range(K):
                        nc.tensor.matmul(
                            cm_psum, lhsT=gate_sb[:, k, b:b + 1],
                            rhs=xin[:, k, h2 * MVW:(h2 + 1) * MVW],
                            start=(k == 0), stop=(k == K - 1))
                    cm_stage = small.tile([1, MVW], f32, tag="cmstage")
                    nc.scalar.copy(out=cm_stage, in_=cm_psum)
                    nc.sync.dma_start(
                        out=cm_dram[b, j * NT + h2 * MVW:
                                    j * NT + (h2 + 1) * MVW],
                        in_=cm_stage)
                # max over c: combine 8 chunks, then transpose 128x128 blocks
                tmax = small.tile([128, NT], f32, tag="tmax")
                nc.vector.reduce_max(out=tmax,
                                     in_=x1.rearrange("p k n -> p n k"),
                                     axis=X)
                for blk in range(BPT):
                    tp = psum.tile([128, 128], f32, tag="tp")
                    nc.tensor.transpose(tp, tmax[:, blk * 128:(blk + 1) * 128],
                                        identity)
                    col = j * BPT + blk
                    nc.vector.reduce_max(out=cx_t[:, b, col:col + 1], in_=tp,
                                         axis=X)

            # ---- softmax over n (transpose layout: [n%128, n//128]) ----
            cm_t = small.tile([128, NB], f32, tag="cmt")
            nc.sync.dma_start(out=cm_t,
                              in_=cm_dram[b].rearrange("(jj p) -> p jj", p=128))
            s_t = small.tile([128, NB], f32, tag="st")
            tmp_t = small.tile([128, NB], f32, tag="st2")
            nc.vector.tensor_scalar_mul(out=s_t, in0=cm_t,
                                        scalar1=params_sb[:, 1:2])
            nc.vector.tensor_scalar_mul(out=tmp_t, in0=cx_t[:, b, :],
                                        scalar1=params_sb[:, 0:1])
            nc.vector.tensor_add(out=s_t, in0=s_t, in1=tmp_t)
            nc.scalar.activation(out=s_t, in_=s_t, func=AF.Relu,
                                 scale=params_sb[:, 2:3],
                                 bias=params_sb[:, 3:4])
            colmax = small.tile([128, 1], f32, tag="cmax")
            nc.vector.reduce_max(out=colmax, in_=s_t, axis=X)
            gmax = small.tile([128, 1], f32, tag="gmax")
            nc.gpsimd.partition_all_reduce(gmax, colmax, 128,
                                           bass_isa.ReduceOp.max)
            ngmax = small.tile([128, 1], f32, tag="ngmax")
            nc.vector.tensor_scalar_mul(out=ngmax, in0=gmax, scalar1=-1.0)
            e_t = small.tile([128, NB], f32, tag="et")
            sume = small.tile([128, 1], f32, tag="sume")
            nc.scalar.activation(out=e_t, in_=s_t, func=AF.Exp, bias=ngmax,
                                 scale=1.0, accum_out=sume)
            gsum = small.tile([128, 1], f32, tag="gsum")
            nc.gpsimd.partition_all_reduce(gsum, sume, 128,
                                           bass_isa.ReduceOp.add)
            rinv = small.tile([128, 1], f32, tag="rinv")
            nc.vector.reciprocal(out=rinv, in_=gsum)
            att_t = small.tile([128, NB], f32, tag="attt")
            nc.vector.tensor_scalar_mul(out=att_t, in0=e_t, scalar1=rinv)
            nc.sync.dma_start(
                out=att_dram[b].rearrange("(jj p) -> p jj", p=128), in_=att_t)

        # ================= pass 3: out = att * gate * x ===================
        for b in range(BC):
            xr = x[b].rearrange("(k p) n -> p k n", p=128)
            outr = out[b].rearrange("(k p) n -> p k n", p=128)
            for j in range(NJ):
                xin = big.tile([128, K, NT], f32, tag="xin")
                nc.sync.dma_start(out=xin, in_=xr[:, :, j * NT:(j + 1) * NT])
                attr = small.tile([128, NT], f32, tag="attr")
                nc.sync.dma_start(
                    out=attr,
                    in_=att_dram[b:b + 1, j * NT:(j + 1) * NT]
                        .to_broadcast((128, NT)))
                yout = big.tile([128, K, NT], f32, tag="big2")
                for k in range(K):
                    nc.vector.tensor_mul(out=yout[:, k, :], in0=xin[:, k, :],
                                         in1=attr)
                    nc.scalar.activation(out=yout[:, k, :], in_=yout[:, k, :],
                                         func=AF.Copy,
                                         scale=gate_sb[:, k, b:b + 1])
                nc.sync.dma_start(out=outr[:, :, j * NT:(j + 1) * NT],
                                  in_=yout)

    nc.compile()
    return nc


def _get_nc(NT=512):
    key = ("nc", NT)
    if key not in _cached:
        _cached[key] = _build_nc(NT)
    return _cached[key]


def _host_params(sw, gamma, beta, running_mean, running_var):
    A = float(gamma[0]) / np.sqrt(float(running_var[0]) + BN_EPS)
    Bconst = float(beta[0]) - float(running_mean[0]) * A
    return np.array([[float(sw[0]), float(sw[1]) / C, A, Bconst]],
                    dtype=np.float32)


def _make_in_maps(x, w1, w2, sw, gamma, beta, running_mean, running_var):
    x = np.ascontiguousarray(np.asarray(x, dtype=np.float32))
    w1t = np.ascontiguousarray(np.asarray(w1, dtype=np.float32).T)
    w2t = np.ascontiguousarray(np.asarray(w2, dtype=np.float32).T)
    params = _host_params(np.asarray(sw), np.asarray(gamma), np.asarray(beta),
                          np.asarray(running_mean), np.asarray(running_var))
    in_maps = []
    for core in range(N_CORES):
        xs = np.ascontiguousarray(x[core * BC:(core + 1) * BC])
        in_maps.append({"x": xs, "w1t": w1t, "w2t": w2t, "params": params})
    return in_maps


def run_sharded(inputs, trace=False, NT=512):
    """Run on all 8 cores; returns (out_full, BassKernelResults)."""
    from concourse.bass_utils import run_bass_kernel_spmd

    nc = _get_nc(NT)
    in_maps = _make_in_maps(**inputs)
    res = run_bass_kernel_spmd(nc, in_maps, core_ids=list(range(N_CORES)),
                               trace=trace)
    out = np.concatenate([r["out"] for r in res.results], axis=0)
    return out, res


def kernel(**inputs) -> np.ndarray:
    out, _ = run_sharded(inputs, trace=False)
    return out
